# revision 4
# baseline (speedup 1.0000x reference)
"""Distributed Trainium2 kernel for nn_Atom91SeqDecoder.

Strategy (per the sharding hint): shard the 8192 residues across the 8
NeuronCores (1024 rows each). Positions X are tiny and replicated, so each
core builds its own [1024, 8192] distance block and exact top-30 kNN — no
halo exchange is needed for graph construction. Per layer, node state is
all-gathered (8 x 0.5 MB) so the kNN gathers (global column indices) stay
local; the small parameter set is replicated.

Key mathematical simplification (verified to 2e-7 against the reference):
the edge-frame rotations R / R^T cancel everywhere. _rot_l1(Rt, f(_rot_l1(
R, x))) with f = channel mixing + per-channel radial gates satisfies
Rt @ R = I, and the attention logits only read the l=0 row which _rot_l1
passes through untouched. The whole network is therefore rotation-free:
no [N,K,3,3] Wigner matrices, no per-edge 3x3 einsums.
"""

import numpy as np

N, K, H, NL, EC, AC, BBC = 8192, 30, 32, 4, 32, 91, 7
HEADS, VC = 8, 16
EPS = 1e-8
BIG = 1e9
NCORES = 8
SH = N // NCORES  # 1024 rows per core


def _normalize(v, jnp, axis=-1):
    return v / jnp.sqrt(jnp.sum(v * v, axis=axis, keepdims=True) + EPS)


def _layer_norm(x, g, b, jnp):
    m = jnp.mean(x, -1, keepdims=True)
    v = jnp.var(x, -1, keepdims=True)
    return (x - m) / jnp.sqrt(v + 1e-5) * g + b


def _norm_so3(emb, g0, g1, jnp):
    l0, l1 = emb[..., 0:1, :], emb[..., 1:4, :]
    l0 = l0 / jnp.sqrt(jnp.mean(l0 * l0, -1, keepdims=True) + EPS) * g0
    n1 = jnp.sqrt(jnp.mean(l1 * l1, (-2, -1), keepdims=True) + EPS)
    return jnp.concatenate([l0, l1 / n1 * g1], axis=-2)


def _dihedrals_noacos(bb, jnp):
    """cos/sin of backbone dihedrals, per-residue form (no flat reshape, no arccos).

    Flat-chain U vectors per residue i: A_i = CA-N, B_i = C-CA, Cv_i = N_{i+1}-C.
    Padded D layout row i = [D(Cv_{i-1},A_i,B_i), D(A_i,B_i,Cv_i), D(B_i,Cv_i,A_{i+1})]
    with out-of-range entries = 0 (cos=1, sin=0).
    """
    n = bb.shape[0]
    Nat, Ca, Cc = bb[:, 0], bb[:, 1], bb[:, 2]
    A = _normalize(Ca - Nat, jnp)                      # [n,3]
    B = _normalize(Cc - Ca, jnp)
    Nat_next = jnp.concatenate([Nat[1:], Nat[-1:]], 0)  # dummy last row
    Cv = _normalize(Nat_next - Cc, jnp)                 # valid i <= n-2
    Cvm1 = jnp.concatenate([Cv[:1], Cv[:-1]], 0)        # Cv_{i-1}; dummy row 0
    Ap1 = jnp.concatenate([A[1:], A[-1:]], 0)           # A_{i+1}; dummy last row

    def f(u2, u1, u0):
        n2 = _normalize(jnp.cross(u2, u1), jnp)
        n1 = _normalize(jnp.cross(u1, u0), jnp)
        c = jnp.clip(jnp.sum(n2 * n1, -1), -1 + 1e-7, 1 - 1e-7)
        w = jnp.sum(u2 * n1, -1)
        s = jnp.sign(w) * jnp.sqrt(jnp.maximum(1.0 - c * c, 0.0))
        return c, s

    c0, s0 = f(Cvm1, A, B)
    c1, s1 = f(A, B, Cv)
    c2, s2 = f(B, Cv, Ap1)
    i = jnp.arange(n)
    first = i == 0
    last = i == n - 1
    c0 = jnp.where(first, 1.0, c0); s0 = jnp.where(first, 0.0, s0)
    c1 = jnp.where(last, 1.0, c1); s1 = jnp.where(last, 0.0, s1)
    c2 = jnp.where(last, 1.0, c2); s2 = jnp.where(last, 0.0, s2)
    return jnp.stack([c0, c1, c2, s0, s1, s2], -1)  # [n,6]


def _orientations(X, jnp):
    z = jnp.zeros((1, 3), X.dtype)
    fwd = jnp.concatenate([_normalize(X[1:] - X[:-1], jnp), z], 0)
    bwd = jnp.concatenate([z, _normalize(X[:-1] - X[1:], jnp)], 0)
    return jnp.stack([fwd, bwd], -2)


def _virtual_cb(bb, jnp):
    Nat, Ca, C = bb[:, 0], bb[:, 1], bb[:, 2]
    b = Ca - Nat
    c = C - Ca
    a = jnp.cross(b, c)
    return -0.58273431 * a + 0.56802827 * b - 0.54067466 * c + Ca


def _bb_embed(bb, jnp):
    X = bb[:, 1]
    dih = jnp.pad(_dihedrals_noacos(bb, jnp), ((0, 0), (0, 1)))  # [N,7]
    bb_rel = bb - X[:, None, :]
    ori = _orientations(X, jnp)
    vcb = (_virtual_cb(bb, jnp) - X)[:, None, :]
    l1 = jnp.nan_to_num(jnp.concatenate([bb_rel, ori, vcb], -2))  # [N,7,3]
    return jnp.concatenate([dih[:, None, :], jnp.swapaxes(l1, -1, -2)], -2)  # [N,4,7]


def _project(emb_full, edge_feat, nbr, p, jnp):
    """Rotation-free project: gather, mix channels, radial gate, mean over K."""
    x = emb_full[nbr]  # [S,K,4,C]
    mix = jnp.concatenate([x[..., 0:1, :] @ p['W0'], x[..., 1:4, :] @ p['W1']], -2)
    r = jnp.maximum(edge_feat @ p['Wr1'] + p['br1'], 0.0) @ p['Wr2'] + p['br2']
    return jnp.mean(mix * r[:, :, None, :], axis=1)


def _build(jax):
    import jax.numpy as jnp
    from jax.sharding import Mesh, PartitionSpec as P
    from jax.experimental.shard_map import shard_map
    from functools import partial

    devs = np.array(jax.devices()[:NCORES])
    mesh = Mesh(devs, ('x',))

    def shard_fn(bb, x_mask, batch, res_emb, seq_features, params):
        # bb, x_mask, batch replicated; res_emb/seq_features sharded on rows.
        ax = jax.lax.axis_index('x')
        row0 = ax * SH
        rows = row0 + jnp.arange(SH)

        X = bb[:, 1]  # [N,3] full
        sq = jnp.sum(X * X, -1)  # [N]
        Xr = jax.lax.dynamic_slice_in_dim(X, row0, SH, 0)  # [SH,3]
        sqr = jax.lax.dynamic_slice_in_dim(sq, row0, SH, 0)
        batch_r = jax.lax.dynamic_slice_in_dim(batch, row0, SH, 0)
        d2 = sqr[:, None] + sq[None, :] - 2.0 * Xr @ X.T  # [SH,N]
        jidx = jnp.arange(N)
        d2 = jnp.where(rows[:, None] == jidx[None, :], BIG, d2)
        d2 = jnp.where(batch_r[:, None] != batch[None, :], BIG, d2)
        d2 = jnp.where(x_mask[None, :], BIG, d2)
        _, nbr = jax.lax.top_k(-d2, K)  # [SH,K] global col indices

        vec = Xr[:, None, :] - X[nbr]
        dist = jnp.sqrt(jnp.sum(vec * vec, -1) + EPS)
        mu = jnp.linspace(0.0, 20.0, 16)
        sig = 20.0 / 16.0
        rbf = jnp.exp(-((dist[..., None] - mu) / sig) ** 2)
        rel = (nbr - rows[:, None]).astype(jnp.float32)
        freq = jnp.exp(jnp.arange(0, 16, 2, dtype=jnp.float32) * (-np.log(10000.0) / 16.0))
        ang = rel[..., None] * freq
        edge_feat = jnp.concatenate([rbf, jnp.cos(ang), jnp.sin(ang)], -1)  # [SH,K,32]

        bb_emb = _bb_embed(bb, jnp)  # [N,4,7] replicated compute (cheap)

        node = _project(bb_emb, edge_feat, nbr, params['embed'], jnp)  # [SH,4,H]
        res, seq = res_emb, seq_features
        for p in params['layers']:
            x = jnp.concatenate([node, res], -1)  # [SH,4,2H]
            inv = jnp.concatenate([x[:, 0, :], seq], -1)
            row0 = (inv @ p['Wf'] + p['bf'])[:, None, :]
            x = jnp.concatenate([row0, x[:, 1:4, :]], axis=1)
            x_full = jax.lax.all_gather(x, 'x', tiled=True)  # [N,4,2H]
            xs = x_full[nbr]  # [SH,K,4,2H]
            dst = jnp.broadcast_to(x[:, None, 0, :], (SH, K, 2 * H))
            a_in = jnp.concatenate([xs[:, :, 0, :], dst, edge_feat], -1)
            logits = jnp.maximum(a_in @ p['Wa1'] + p['ba1'], 0.0) @ p['Wa2'] + p['ba2']
            attn = jax.nn.softmax(logits, axis=1)
            v = (xs @ p['Wv']).reshape(SH, K, 4, HEADS, VC)
            node = jnp.einsum('nkh,nkmhv->nmhv', attn, v).reshape(SH, 4, HEADS * VC) @ p['Wo']
            ru = jnp.concatenate([node[:, 0:1, :] @ p['Wr0'] + p['br0'],
                                  node[:, 1:4, :] @ p['Wr1l']], -2)
            res = _norm_so3(res + ru, p['g0'], p['g1'], jnp)
            seq = _layer_norm(seq + node[:, 0, :] @ p['Ws'] + p['bs'], p['lg'], p['lb'], jnp)
            inv_n = node[:, 0, :]
            inv_full = jax.lax.all_gather(inv_n, 'x', tiled=True)  # [N,H]
            e_in = jnp.concatenate([inv_full[nbr], jnp.broadcast_to(inv_n[:, None, :], (SH, K, H)),
                                    edge_feat], -1)
            edge_feat = edge_feat + jnp.maximum(e_in @ p['We1'] + p['be1'], 0.0) @ p['We2'] + p['be2']

        res = _norm_so3(res, params['fg0'], params['fg1'], jnp)
        res_full = jax.lax.all_gather(res, 'x', tiled=True)  # [N,4,H]
        atom = _project(res_full, edge_feat, nbr, params['atoms'], jnp)  # [SH,4,AC]
        latent = jnp.swapaxes(atom[:, 1:4, :], -1, -2)  # [SH,AC,3]

        hp = params['head']
        h = _layer_norm(seq_features, hp['g'], hp['b'], jnp)
        h = jnp.maximum(h @ hp['W1'] + hp['b1'], 0.0)
        h = jnp.maximum(h @ hp['W2'] + hp['b2'], 0.0)
        lg = jax.nn.log_softmax(h @ hp['W3'] + hp['b3'], axis=-1)  # [SH,20]
        return latent, lg

    pspec_params = None  # replicated pytree
    fn = shard_map(
        shard_fn, mesh=mesh,
        in_specs=(P(), P(), P(), P('x'), P('x'), P()),
        out_specs=(P('x'), P('x')),
        check_rep=False,
    )
    return jax.jit(fn), mesh


_COMPILED = {}


def kernel(bb, x_mask, batch, res_emb, seq_features, params):
    import jax

    if 'fn' not in _COMPILED:
        _COMPILED['fn'], _COMPILED['mesh'] = _build(jax)
    fn = _COMPILED['fn']

    bb = np.asarray(bb, np.float32)
    x_mask = np.asarray(x_mask)
    batch = np.asarray(batch, np.int32)
    res_emb = np.asarray(res_emb, np.float32)
    seq_features = np.asarray(seq_features, np.float32)

    latent, logits = fn(bb, x_mask, batch, res_emb, seq_features, params)
    latent = np.asarray(jax.device_get(latent), np.float32)
    logits = np.asarray(jax.device_get(logits), np.float32)
    return latent, logits


if __name__ == '__main__':
    pass
